# revision 2
# baseline (speedup 1.0000x reference)
"""Trainium2 Bass kernel for nn_Generator_LSTM_23433341567859.

Computes: init LSTM (tanh) over [B=65536, T=32, F=16] -> final (h, c),
batchnorm h and c (training stats over the full batch), 14 autoregressive
LSTM (relu) steps feeding h to itself, reverse, batchnorm, backward LSTM
(relu, return_sequences) -> [B, 14, 4].

Sharding: pure data parallel over batch across 8 NeuronCores; tiny
weights replicated; BN mean/var partial sums all-reduced across cores.

Device layout (per core, B_c = 8192):
  batch is split into S=16 subtiles of N=512; partition p = s*8 + c
  (s = subtile, c = channel). LSTM gate pre-activations are computed
  class-major (i/f/g/o), each as a [128, 512] PSUM tile, via block-
  diagonal weight matmuls:
     z_cls = WA_cls.T @ XA + WB_cls.T @ XB + U_cls.T @ H
  (K=128 contraction = 16 subtiles x 8 features; X split in two feature
  halves since 16 feats x 16 subtiles > 128 partitions).
  All elementwise work is full-lane [128, 512]. Stage 3 (Do=4) uses
  class-major [64, 512].

Host does layout staging only: input transpose into DMA-friendly slabs,
blockdiagonal weight construction, and the inverse output transpose.
"""

import sys

import ml_dtypes
import numpy as np

BF16 = ml_dtypes.bfloat16

for _p in ("/opt/trn_rl_repo", "/opt/trn_rl_repo/concourse"):
    if _p not in sys.path:
        sys.path.insert(0, _p)

B = 65536
T = 32
F = 16
D = 8
DO = 4
ROWS = 14
NCORES = 8
BC = B // NCORES          # 8192 batch rows per core
S = 16                    # subtiles per core
N = BC // S               # 512 batch cols per subtile
EPS = 1e-3
F32 = np.float32


# ----------------------------------------------------------------------------
# host-side weight/layout staging
# ----------------------------------------------------------------------------

def _blockdiag(sub: np.ndarray, s: int) -> np.ndarray:
    k0, m0 = sub.shape
    out = np.zeros((s, k0, s, m0), F32)
    idx = np.arange(s)
    out[idx, :, idx, :] = sub[None]
    return out.reshape(s * k0, s * m0)


def _rep_chan(vec: np.ndarray, nchan: int) -> np.ndarray:
    """[nchan] -> [128] replicated so row p holds vec[p % nchan]."""
    return np.tile(np.asarray(vec, F32), 128 // nchan)


def _prep_x(noise_core: np.ndarray) -> np.ndarray:
    """[8192, 32, 16] -> [16, 128, 2048] t-pair DMA slabs.

    Slab i columns: [XA(2i) | XB(2i) | XA(2i+1) | XB(2i+1)], each 512 wide.
    XA_t[p = s*8+f, n] = x[s*512+n, t, f]      (features 0..7)
    XB_t likewise for features 8..15.
    """
    a = noise_core.reshape(S, N, T, F).transpose(2, 0, 3, 1)   # [t, s, f, n]
    xa = a[:, :, 0:8, :].reshape(T, 128, N)
    xb = a[:, :, 8:16, :].reshape(T, 128, N)
    x6 = np.stack([xa, xb], axis=1)                            # [32, 2, 128, 512]
    x7 = x6.reshape(T // 2, 2, 2, 128, N).transpose(0, 3, 1, 2, 4)
    return np.ascontiguousarray(x7.reshape(T // 2, 128, 4 * N).astype(BF16))


def _unshuffle_out(raw: np.ndarray) -> np.ndarray:
    """[64, 14*512] device layout -> [8192, 14, 4]."""
    a = raw.astype(F32).reshape(S, DO, ROWS, N)            # [s, c, r, n]
    return np.ascontiguousarray(a.transpose(0, 3, 2, 1).reshape(BC, ROWS, DO))


# ----------------------------------------------------------------------------
# device program
# ----------------------------------------------------------------------------

def _build_program():
    from concourse import bacc, tile, mybir

    f32 = mybir.dt.float32
    bf16 = mybir.dt.bfloat16
    AF = mybir.ActivationFunctionType
    OP = mybir.AluOpType
    AX = mybir.AxisListType

    nc = bacc.Bacc(
        "TRN2",
        target_bir_lowering=False,
        debug=False,
        enable_asserts=False,
        num_devices=NCORES,
    )

    xd = nc.dram_tensor("x", [T // 2, 128, 4 * N], bf16, kind="ExternalInput").ap()
    wa_d = nc.dram_tensor("wa", [4, 128, 128], bf16, kind="ExternalInput").ap()
    wb_d = nc.dram_tensor("wb", [4, 128, 128], bf16, kind="ExternalInput").ap()
    uu_d = nc.dram_tensor("uu", [4, 128, 128], bf16, kind="ExternalInput").ap()
    v1_d = nc.dram_tensor("v1", [4, 128, 128], bf16, kind="ExternalInput").ap()
    w2_d = nc.dram_tensor("w2", [4, 128, 64], bf16, kind="ExternalInput").ap()
    u2_d = nc.dram_tensor("u2", [4, 64, 64], bf16, kind="ExternalInput").ap()
    ba_d = nc.dram_tensor("ball", [128, 18], f32, kind="ExternalInput").ap()
    sel_d = nc.dram_tensor("selrep", [128, 128], f32, kind="ExternalInput").ap()
    out_d = nc.dram_tensor("out_sbc", [64, ROWS * N], bf16, kind="ExternalOutput").ap()

    with tile.TileContext(nc) as tc:
        with (
            tc.tile_pool(name="wpool", bufs=1) as wp,
            tc.tile_pool(name="persist", bufs=1) as st,
            tc.tile_pool(name="xpool", bufs=4) as xp,
            tc.tile_pool(name="spool", bufs=3) as sp,
            tc.tile_pool(name="zpool", bufs=8, space="PSUM") as zp,
            tc.tile_pool(name="dram", bufs=1, space="DRAM") as dp,
        ):
            # ---- load weights / constants (one DMA per group) ----
            def wgroup(name, dram, n, cols):
                t = wp.tile([128, n * cols], bf16, name=name)
                nc.gpsimd.dma_start(t[:], dram.rearrange("g p m -> p g m"))
                return [t[:, i * cols:(i + 1) * cols] for i in range(n)]

            wa = wgroup("wag", wa_d, 4, 128)
            wb = wgroup("wbg", wb_d, 4, 128)
            uu = wgroup("uug", uu_d, 4, 128)
            v1 = wgroup("v1g", v1_d, 4, 128)
            w2 = wgroup("w2g", w2_d, 4, 64)
            u2g = wp.tile([64, 4 * 64], bf16, name="u2g")
            nc.gpsimd.dma_start(u2g[:], u2_d.rearrange("g p m -> p g m"))
            u2 = [u2g[:, i * 64:(i + 1) * 64] for i in range(4)]
            ball = wp.tile([128, 18], f32, name="ball")
            nc.gpsimd.dma_start(ball[:], ba_d[:])
            b1s = ball[:, 0:4]
            b2s = ball[:, 4:8]
            b3s = ball[:, 8:12]
            bnp = ball[:, 12:18]
            sel = wp.tile([128, 128], f32, name="sel")
            nc.gpsimd.dma_start(sel[:], sel_d[:])
            zeroT = wp.tile([128, 1], f32, name="zeroT")
            nc.gpsimd.memset(zeroT[:], 0.0)
            epsT = wp.tile([128, 1], f32, name="epsT")
            nc.gpsimd.memset(epsT[:], EPS)

            # warm up the collectives firmware path early; result unused
            cw_in = dp.tile([128, 1], f32, name="cw_in")
            cw_out = dp.tile([128, 1], f32, name="cw_out", addr_space="Shared")
            warmz = wp.tile([128, 1], f32, name="warmz")
            nc.gpsimd.memset(warmz[:], 0.0)
            nc.gpsimd.dma_start(cw_in[:], warmz[:])
            nc.gpsimd.collective_compute(
                "AllReduce", OP.add,
                replica_groups=[list(range(NCORES))],
                ins=[cw_in.opt()], outs=[cw_out.opt()],
            )

            # ---- persistent state ----
            C = st.tile([128, N], f32, name="C")
            H = st.tile([128, N], bf16, name="H")
            HS = st.tile([128, ROWS * N], bf16, name="HS")
            OUT = st.tile([64, ROWS * N], bf16, name="OUT")
            C3 = st.tile([64, N], f32, name="C3")
            st4 = st.tile([128, 4], f32, name="st4")
            st3 = st.tile([128, 2], f32, name="st3")
            sums2 = st.tile([128, ROWS], f32, name="sums2")
            sqs2 = st.tile([128, ROWS], f32, name="sqs2")
            scr = st.tile([128, N], f32, name="scr")

            # =========================== stage 1 ===========================
            # Software-pipelined: step t+1's X-matmuls (H-independent) are
            # issued BEFORE step t's U-matmuls so PE always has queued work
            # while the elementwise chain produces H_t. 8 PSUM banks hold
            # z(t) (closing) + z(t+1) (X accumulating).
            ORD = [1, 2, 0, 3]            # f first (P2 path), then g
            xs_tiles = {}

            def load_slab(i):
                xs = xp.tile([128, 4 * N], bf16, name="xs")
                if i == 0:
                    for j in range(4):
                        nc.sync.dma_start(xs[:, j * N:(j + 1) * N],
                                          xd[i, :, j * N:(j + 1) * N])
                else:
                    nc.sync.dma_start(xs[:], xd[i])
                xs_tiles[i] = xs

            def xab(t):
                xs = xs_tiles[t // 2]
                d = t % 2
                return (xs[:, (2 * d) * N:(2 * d + 1) * N],
                        xs[:, (2 * d + 1) * N:(2 * d + 2) * N])

            load_slab(0)
            load_slab(1)
            zcur = {}
            XA, XB = xab(0)
            for ci in ORD:
                zc = zp.tile([128, N], f32, name=f"z{ci}", tag="z")
                nc.tensor.matmul(zc[:], wa[ci], XA, start=True, stop=False)
                nc.tensor.matmul(zc[:], wb[ci], XB, start=False, stop=True)
                zcur[ci] = zc

            for t in range(T):
                z = zcur
                if t + 1 < T:
                    s = t + 1
                    if s % 2 == 0 and s // 2 + 1 < T // 2:
                        load_slab(s // 2 + 1)
                    XA, XB = xab(s)
                    zn = {}
                    for ci in ORD:
                        zc = zp.tile([128, N], f32, name=f"z{ci}", tag="z")
                        nc.tensor.matmul(zc[:], wa[ci], XA, start=True, stop=False)
                        nc.tensor.matmul(zc[:], wb[ci], XB, start=False, stop=False)
                        zn[ci] = zc
                    zcur = zn
                if t > 0:
                    for ci in ORD:
                        nc.tensor.matmul(z[ci][:], uu[ci], H[:],
                                         start=False, stop=True)
                Sf = sp.tile([128, N], f32, name="Sf")
                nc.scalar.activation(Sf[:], z[1][:], AF.Sigmoid, bias=b1s[:, 1:2])
                if t > 0:
                    P2 = sp.tile([128, N], f32, name="P2")
                    nc.vector.tensor_mul(P2[:], Sf[:], C[:])
                Gt = sp.tile([128, N], bf16, name="Gt")
                nc.scalar.activation(Gt[:], z[2][:], AF.Tanh, bias=b1s[:, 2:3])
                Si = sp.tile([128, N], bf16, name="Si")
                nc.scalar.activation(Si[:], z[0][:], AF.Sigmoid, bias=b1s[:, 0:1])
                So = sp.tile([128, N], bf16, name="So")
                nc.scalar.activation(So[:], z[3][:], AF.Sigmoid, bias=b1s[:, 3:4])
                if t == 0:
                    nc.vector.tensor_mul(C[:], Si[:], Gt[:])
                else:
                    P1 = sp.tile([128, N], bf16, name="P1")
                    nc.vector.tensor_mul(P1[:], Si[:], Gt[:])
                    nc.vector.tensor_add(C[:], P1[:], P2[:])
                Ct = sp.tile([128, N], bf16, name="Ct")
                nc.scalar.activation(Ct[:], C[:], AF.Tanh, bias=zeroT[:, 0:1])
                nc.vector.tensor_mul(H[:], So[:], Ct[:])

            # ---- stage-1 BN stats: per-partition sums, then per-channel ----
            # st4 cols: (h_sum, c_sum, h_sumsq, c_sumsq)
            nc.vector.tensor_reduce(st4[:, 0:1], H[:], axis=AX.X, op=OP.add)
            nc.vector.tensor_reduce(st4[:, 1:2], C[:], axis=AX.X, op=OP.add)
            nc.scalar.activation(scr[:], H[:], AF.Square, bias=zeroT[:, 0:1], accum_out=st4[:, 2:3])
            nc.scalar.activation(scr[:], C[:], AF.Square, bias=zeroT[:, 0:1], accum_out=st4[:, 3:4])
            gp = zp.tile([128, 4], f32, name="gp", tag="z")
            nc.tensor.matmul(gp[:], sel[:], st4[:], start=True, stop=True)
            gs = st.tile([128, 4], f32, name="gs")
            nc.scalar.copy(gs[:], gp[:])

            cin1 = dp.tile([128, 4], f32, name="cin1")
            cout1 = dp.tile([128, 4], f32, name="cout1", addr_space="Shared")
            nc.sync.dma_start(cin1[:], gs[:])
            nc.gpsimd.collective_compute(
                "AllReduce", OP.add,
                replica_groups=[list(range(NCORES))],
                ins=[cin1.opt()], outs=[cout1.opt()],
            )
            gg = st.tile([128, 4], f32, name="gg")
            nc.sync.dma_start(gg[:], cout1[:])

            # batched per-channel affine for h and c: x_bn = a*x + be
            m = st.tile([128, 2], f32, name="m")
            nc.vector.tensor_scalar(m[:], gg[:, 0:2], 1.0 / float(B), None,
                                    op0=OP.mult)
            q = st.tile([128, 2], f32, name="q")
            nc.vector.tensor_scalar(q[:], gg[:, 2:4], 1.0 / float(B), None,
                                    op0=OP.mult)
            m2 = st.tile([128, 2], f32, name="m2")
            nc.vector.tensor_mul(m2[:], m[:], m[:])
            v = st.tile([128, 2], f32, name="v")
            nc.vector.tensor_sub(v[:], q[:], m2[:])
            sd = st.tile([128, 2], f32, name="sd")
            nc.scalar.activation(sd[:], v[:], AF.Sqrt, bias=epsT[:, 0:1])
            iv = st.tile([128, 2], f32, name="iv")
            nc.vector.reciprocal(iv[:], sd[:])
            a2 = st.tile([128, 2], f32, name="a2")
            nc.vector.tensor_mul(a2[:], bnp[:, 0:2], iv[:])
            na2 = st.tile([128, 2], f32, name="na2")
            nc.vector.tensor_scalar(na2[:], a2[:], -1.0, None, op0=OP.mult)
            # be = beta - m*a   (computed per column pair)
            be2 = st.tile([128, 2], f32, name="be2")
            nc.vector.tensor_mul(be2[:], m[:], na2[:])
            nc.vector.tensor_add(be2[:], be2[:], bnp[:, 2:4])

            # materialize BN(h) once (bf16, feeds stage-2 r=0 matmuls)
            Hb = st.tile([128, N], bf16, name="Hb")
            nc.vector.tensor_scalar(Hb[:], H[:], a2[:, 0:1], be2[:, 0:1],
                                    op0=OP.mult, op1=OP.add)
            # BN(c) applied in place
            nc.vector.tensor_scalar(C[:], C[:], a2[:, 1:2], be2[:, 1:2],
                                    op0=OP.mult, op1=OP.add)

            # =========================== stage 2 ===========================
            for r in range(ROWS):
                hin = Hb[:] if r == 0 else HS[:, (r - 1) * N:r * N]
                z2 = [None] * 4
                for ci in [1, 2, 0, 3]:
                    zc = zp.tile([128, N], f32, name=f"z2_{ci}", tag="z")
                    nc.tensor.matmul(zc[:], v1[ci], hin, start=True, stop=True)
                    z2[ci] = zc
                bias = b2s
                Sf = sp.tile([128, N], f32, name="Sf2")
                nc.scalar.activation(Sf[:], z2[1][:], AF.Sigmoid, bias=bias[:, 1:2])
                P2 = sp.tile([128, N], f32, name="P2b")
                nc.vector.tensor_mul(P2[:], Sf[:], C[:])
                Gr = sp.tile([128, N], bf16, name="Gr2")
                nc.vector.tensor_scalar(Gr[:], z2[2][:], bias[:, 2:3], 0.0,
                                        op0=OP.add, op1=OP.max)
                Si = sp.tile([128, N], bf16, name="Si2")
                nc.scalar.activation(Si[:], z2[0][:], AF.Sigmoid, bias=bias[:, 0:1])
                So = sp.tile([128, N], bf16, name="So2")
                nc.scalar.activation(So[:], z2[3][:], AF.Sigmoid, bias=bias[:, 3:4])
                P1 = sp.tile([128, N], bf16, name="P1b")
                nc.vector.tensor_mul(P1[:], Si[:], Gr[:])
                nc.vector.tensor_add(C[:], P1[:], P2[:])
                hs_sl = HS[:, r * N:(r + 1) * N]
                nc.vector.scalar_tensor_tensor(
                    hs_sl, C[:], 0.0, So[:], op0=OP.max, op1=OP.mult,
                    accum_out=sums2[:, r:r + 1])
                nc.vector.scalar_tensor_tensor(
                    scr[:], hs_sl, 0.0, hs_sl, op0=OP.add, op1=OP.mult,
                    accum_out=sqs2[:, r:r + 1])

            # ---- BN3 stats over (batch, rows) ----
            nc.vector.tensor_reduce(st3[:, 0:1], sums2[:], axis=AX.X, op=OP.add)
            nc.vector.tensor_reduce(st3[:, 1:2], sqs2[:], axis=AX.X, op=OP.add)
            gp3 = zp.tile([128, 2], f32, name="gp3", tag="z")
            nc.tensor.matmul(gp3[:], sel[:], st3[:], start=True, stop=True)
            gs3 = st.tile([128, 2], f32, name="gs3")
            nc.scalar.copy(gs3[:], gp3[:])
            cin3 = dp.tile([128, 2], f32, name="cin3")
            cout3 = dp.tile([128, 2], f32, name="cout3", addr_space="Shared")
            nc.sync.dma_start(cin3[:], gs3[:])
            nc.gpsimd.collective_compute(
                "AllReduce", OP.add,
                replica_groups=[list(range(NCORES))],
                ins=[cin3.opt()], outs=[cout3.opt()],
            )
            gg3 = st.tile([128, 2], f32, name="gg3")
            nc.sync.dma_start(gg3[:], cout3[:])

            n3 = float(B) * ROWS
            m3 = st.tile([128, 1], f32, name="m3")
            nc.vector.tensor_scalar(m3[:], gg3[:, 0:1], 1.0 / n3, None, op0=OP.mult)
            q3 = st.tile([128, 1], f32, name="q3")
            nc.vector.tensor_scalar(q3[:], gg3[:, 1:2], 1.0 / n3, None, op0=OP.mult)
            m32 = st.tile([128, 1], f32, name="m32")
            nc.vector.tensor_mul(m32[:], m3[:], m3[:])
            v3 = st.tile([128, 1], f32, name="v3")
            nc.vector.tensor_sub(v3[:], q3[:], m32[:])
            sd3 = st.tile([128, 1], f32, name="sd3")
            nc.scalar.activation(sd3[:], v3[:], AF.Sqrt, bias=epsT[:, 0:1])
            iv3 = st.tile([128, 1], f32, name="iv3")
            nc.vector.reciprocal(iv3[:], sd3[:])
            a3 = st.tile([128, 1], f32, name="a3")
            nc.vector.tensor_mul(a3[:], bnp[:, 4:5], iv3[:])
            na3 = st.tile([128, 1], f32, name="na3")
            nc.vector.tensor_scalar(na3[:], a3[:], -1.0, None, op0=OP.mult)
            b3e = st.tile([128, 1], f32, name="b3e")
            nc.vector.scalar_tensor_tensor(
                b3e[:], m3[:], na3[:, 0:1], bnp[:, 5:6], op0=OP.mult, op1=OP.add)

            # fold BN3 into stage-3 input weights: w2s = diag(a3) @ w2
            w2s = []
            for ci in range(4):
                wt = st.tile([128, 64], bf16, name=f"w2s{ci}")
                nc.vector.tensor_scalar(wt[:], w2[ci], a3[:, 0:1], None,
                                        op0=OP.mult)
                w2s.append(wt)
            b3e_b = st.tile([128, 1], bf16, name="b3e_b")
            nc.vector.tensor_copy(b3e_b[:], b3e[:])
            b3eff = st.tile([64, 4], f32, name="b3eff")
            for ci in range(4):
                bp3 = zp.tile([64, 1], f32, name="bp3", tag="z")
                nc.tensor.matmul(bp3[:], w2[ci], b3e_b[:], start=True, stop=True)
                nc.vector.tensor_add(b3eff[:, ci:ci + 1], bp3[:], b3s[0:64, ci:ci + 1])

            # =========================== stage 3 ===========================
            for r in range(ROWS):
                xr = HS[:, (ROWS - 1 - r) * N:(ROWS - r) * N]
                z3 = [None] * 4
                for ci in [1, 2, 0, 3]:
                    zc = zp.tile([64, N], f32, name=f"z3_{ci}", tag="z")
                    nc.tensor.matmul(zc[:], w2s[ci][:], xr,
                                     start=True, stop=(r == 0))
                    if r > 0:
                        nc.tensor.matmul(zc[:], u2[ci],
                                         OUT[:, (r - 1) * N:r * N],
                                         start=False, stop=True)
                    z3[ci] = zc
                Sf = sp.tile([64, N], f32, name="Sf3")
                nc.scalar.activation(Sf[:], z3[1][:], AF.Sigmoid, bias=b3eff[:, 1:2])
                if r > 0:
                    P2 = sp.tile([64, N], f32, name="P23")
                    nc.vector.tensor_mul(P2[:], Sf[:], C3[:])
                Gr = sp.tile([64, N], bf16, name="Gr3")
                nc.vector.tensor_scalar(Gr[:], z3[2][:], b3eff[:, 2:3], 0.0,
                                        op0=OP.add, op1=OP.max)
                Si = sp.tile([64, N], bf16, name="Si3")
                nc.scalar.activation(Si[:], z3[0][:], AF.Sigmoid, bias=b3eff[:, 0:1])
                So = sp.tile([64, N], bf16, name="So3")
                nc.scalar.activation(So[:], z3[3][:], AF.Sigmoid, bias=b3eff[:, 3:4])
                if r == 0:
                    nc.vector.tensor_mul(C3[:], Si[:], Gr[:])
                else:
                    P1 = sp.tile([64, N], bf16, name="P13")
                    nc.vector.tensor_mul(P1[:], Si[:], Gr[:])
                    nc.vector.tensor_add(C3[:], P1[:], P2[:])
                nc.vector.scalar_tensor_tensor(
                    OUT[:, r * N:(r + 1) * N], C3[:], 0.0, So[:],
                    op0=OP.max, op1=OP.mult)
                nc.sync.dma_start(out_d[:, r * N:(r + 1) * N],
                                  OUT[:, r * N:(r + 1) * N])

    nc.compile()
    return nc


# ----------------------------------------------------------------------------
# entry point
# ----------------------------------------------------------------------------

_CACHED = {}


def _get_program():
    if "nc" not in _CACHED:
        _CACHED["nc"] = _build_program()
    return _CACHED["nc"]


def kernel(noise_seed, W0, U0, b0, gamma_h, beta_h, gamma_c, beta_c,
           W1, U1, b1, gamma3, beta3, W2, U2, b2, training=1, **_):
    from concourse.bass_utils import run_bass_kernel_spmd

    noise_seed = np.ascontiguousarray(np.asarray(noise_seed, F32))
    W0 = np.asarray(W0, F32); U0 = np.asarray(U0, F32); b0 = np.asarray(b0, F32)
    W1 = np.asarray(W1, F32); U1 = np.asarray(U1, F32); b1 = np.asarray(b1, F32)
    W2 = np.asarray(W2, F32); U2 = np.asarray(U2, F32); b2 = np.asarray(b2, F32)
    assert noise_seed.shape == (B, T, F)

    # class column ranges in keras gate order i, f, g, o
    cls0 = [W0[:, 8 * k:8 * (k + 1)] for k in range(4)]
    u0c = [U0[:, 8 * k:8 * (k + 1)] for k in range(4)]
    V1 = (W1 + U1)
    v1c = [V1[:, 8 * k:8 * (k + 1)] for k in range(4)]
    w2c = [W2[:, 4 * k:4 * (k + 1)] for k in range(4)]
    u2c = [U2[:, 4 * k:4 * (k + 1)] for k in range(4)]

    wa = np.stack([_blockdiag(c[0:8], S) for c in cls0])            # [4,128,128]
    wb = np.stack([_blockdiag(c[8:16], S) for c in cls0])
    uu = np.stack([_blockdiag(c, S) for c in u0c])
    v1 = np.stack([_blockdiag(c, S) for c in v1c])
    w2 = np.stack([_blockdiag(c, S) for c in w2c])                  # [4,128,64]
    u2 = np.stack([_blockdiag(c, S) for c in u2c])                  # [4,64,64]

    b1s = np.stack([_rep_chan(b0[8 * k:8 * (k + 1)], 8) for k in range(4)], 1)
    b2s = np.stack([_rep_chan(b1[8 * k:8 * (k + 1)], 8) for k in range(4)], 1)
    b3s = np.stack([_rep_chan(b2[4 * k:4 * (k + 1)], 4) for k in range(4)], 1)
    bnp = np.stack([_rep_chan(v, 8) for v in
                    (gamma_h, gamma_c, beta_h, beta_c, gamma3, beta3)], 1)
    ball = np.concatenate([b1s, b2s, b3s, bnp], 1).astype(F32)
    p = np.arange(128)
    selrep = (p[:, None] % 8 == p[None, :] % 8).astype(F32)

    shared = {
        "wa": np.ascontiguousarray(wa.astype(BF16)),
        "wb": np.ascontiguousarray(wb.astype(BF16)),
        "uu": np.ascontiguousarray(uu.astype(BF16)),
        "v1": np.ascontiguousarray(v1.astype(BF16)),
        "w2": np.ascontiguousarray(w2.astype(BF16)),
        "u2": np.ascontiguousarray(u2.astype(BF16)),
        "ball": np.ascontiguousarray(ball),
        "selrep": selrep,
    }
    in_maps = []
    for c in range(NCORES):
        m = dict(shared)
        m["x"] = _prep_x(noise_seed[c * BC:(c + 1) * BC])
        in_maps.append(m)

    nc = _get_program()
    res = run_bass_kernel_spmd(nc, in_maps, list(range(NCORES)))
    _CACHED["last_exec_time_ns"] = res.exec_time_ns
    _CACHED["last_results"] = res
    out = np.empty((B, ROWS, DO), F32)
    for c in range(NCORES):
        out[c * BC:(c + 1) * BC] = _unshuffle_out(res.results[c]["out_sbc"])
    return out


if __name__ == "__main__":
    rng = np.random.default_rng(0)
    inputs = {
        "noise_seed": rng.standard_normal((B, T, F), dtype=F32),
        "W0": 0.1 * rng.standard_normal((F, 4 * D), dtype=F32),
        "U0": 0.1 * rng.standard_normal((D, 4 * D), dtype=F32),
        "b0": np.zeros(4 * D, F32),
        "gamma_h": np.ones(D, F32), "beta_h": np.zeros(D, F32),
        "gamma_c": np.ones(D, F32), "beta_c": np.zeros(D, F32),
        "W1": 0.1 * rng.standard_normal((D, 4 * D), dtype=F32),
        "U1": 0.1 * rng.standard_normal((D, 4 * D), dtype=F32),
        "b1": np.zeros(4 * D, F32),
        "gamma3": np.ones(D, F32), "beta3": np.zeros(D, F32),
        "W2": 0.1 * rng.standard_normal((D, 4 * DO), dtype=F32),
        "U2": 0.1 * rng.standard_normal((DO, 4 * DO), dtype=F32),
        "b2": np.zeros(4 * DO, F32),
        "training": 1,
    }
    out = kernel(**inputs)
    print("kernel out", out.shape, out.dtype, float(np.abs(out).mean()))

